# revision 6
# baseline (speedup 1.0000x reference)
"""Trainium2 Bass kernel for nn_ASPP (gather-GEMM sparse conv x3 + BN + ReLU + sum).

Architecture:
  - Shard voxels (rows of feats / columns of nbr maps) across 8 NeuronCores.
  - Per core, a Bass/Tile kernel computes the three sparse-conv outputs
    x_b = sum_k feats_pad[nbr_b[k, v]] @ W_b[k]  for its voxel shard,
    channel-major [64, Nc], using:
      * indirect DMA row-gathers from an HBM-resident feats_pad table
        (128 rows per call, int32 offsets),
      * PE transpose of gathered [128 vox, 2k x 64ch] pair-blocks,
      * paired matmuls accumulating in PSUM per branch.
  - BatchNorm statistics (mean/var over all N voxels) + ReLU + branch sum are
    done by XLA around the Bass custom call, with jax.lax.psum across the 8
    cores inside one shard_map jit (single device round trip).

The kernel is built and jitted once per process and cached.
"""

import numpy as np

import jax
import jax.numpy as jnp
from jax.sharding import Mesh, PartitionSpec as P, NamedSharding
from jax.experimental.shard_map import shard_map

import concourse.bass as bass
import concourse.mybir as mybir
import concourse.tile as tile
from concourse import bacc
from concourse.bass import ds
from concourse.bass2jax import _bass_exec_p, install_neuronx_cc_hook, partition_id_tensor
from concourse.masks import make_identity

NCORES = 8
N = 100000
C = 64
EPS = 1e-5

# branch k-counts padded to even so 2-k pairs never straddle a branch
K_RAW = (27, 125, 343)
K_PAD = (28, 126, 344)
KTOT = sum(K_PAD)            # 498
NPAIR = KTOT // 2            # 249
# pair index ranges per branch: [0,14), [14,77), [77,249)
PAIR_LO = (0, K_PAD[0] // 2, (K_PAD[0] + K_PAD[1]) // 2)
PAIR_HI = (K_PAD[0] // 2, (K_PAD[0] + K_PAD[1]) // 2, NPAIR)

NC_SHARD = 12544             # per-core voxels (98 * 128); 8*12544 = 100352 >= N
NTILE = NC_SHARD // 128      # 98


def _build_bass():
    """Per-core conv kernel: inputs table [N+1, C] f32, wp [128, NPAIR*64] f32,
    idx [NC_SHARD, KTOT] int32; outputs x1, x2, x3 [64, NC_SHARD] f32."""
    nc = bacc.Bacc("TRN2", target_bir_lowering=False, debug=False, num_devices=NCORES)

    table = nc.dram_tensor("table", [N + 1, C], mybir.dt.float32, kind="ExternalInput")
    wp = nc.dram_tensor("wp", [128, NPAIR * C], mybir.dt.float32, kind="ExternalInput")
    idx = nc.dram_tensor("idx", [NC_SHARD, KTOT], mybir.dt.int32, kind="ExternalInput")
    xs = [
        nc.dram_tensor(f"x{b+1}", [C, NC_SHARD], mybir.dt.float32, kind="ExternalOutput")
        for b in range(3)
    ]

    with tile.TileContext(nc) as tc:
        with (
            tc.tile_pool(name="const", bufs=1) as constp,
            tc.tile_pool(name="idxp", bufs=2) as idxp,
            tc.tile_pool(name="gp", bufs=40) as gpool,
            tc.tile_pool(name="gtp", bufs=8) as gtp,
            tc.tile_pool(name="xbp", bufs=4) as xbp,
            tc.tile_pool(name="ps", bufs=3, space="PSUM") as psp,
            tc.tile_pool(name="accp", bufs=3, space="PSUM") as accp,
        ):
            wsb = constp.tile([128, NPAIR * C], mybir.dt.float32)
            nc.sync.dma_start(out=wsb[:], in_=wp[:])
            ident = constp.tile([128, 128], mybir.dt.float32)
            make_identity(nc, ident[:])

            with tc.For_i(
                0, NC_SHARD, 128,
                hint_engines=(mybir.EngineType.Pool, mybir.EngineType.PE),
            ) as i:
                idx_t = idxp.tile([128, KTOT], mybir.dt.int32, tag="idx")
                nc.sync.dma_start(out=idx_t[:], in_=idx[ds(i, 128)])

                for b in range(3):
                    acc = accp.tile([64, 128], mybir.dt.float32, tag="acc")
                    for p in range(PAIR_LO[b], PAIR_HI[b]):
                        gt = gpool.tile([128, 128], mybir.dt.float32, tag="g")
                        nc.gpsimd.indirect_dma_start(
                            out=gt[:, 0:C],
                            out_offset=None,
                            in_=table[:],
                            in_offset=bass.IndirectOffsetOnAxis(
                                ap=idx_t[:, 2 * p : 2 * p + 1], axis=0
                            ),
                        )
                        nc.gpsimd.indirect_dma_start(
                            out=gt[:, C : 2 * C],
                            out_offset=None,
                            in_=table[:],
                            in_offset=bass.IndirectOffsetOnAxis(
                                ap=idx_t[:, 2 * p + 1 : 2 * p + 2], axis=0
                            ),
                        )
                        tp = psp.tile([128, 128], mybir.dt.float32, tag="tp")
                        nc.tensor.transpose(out=tp[:], in_=gt[:], identity=ident[:])
                        gts = gtp.tile([128, 128], mybir.dt.float32, tag="gts")
                        if p % 2 == 0:
                            nc.vector.tensor_copy(out=gts[:], in_=tp[:])
                        else:
                            nc.scalar.copy(out=gts[:], in_=tp[:])
                        nc.tensor.matmul(
                            out=acc[:],
                            lhsT=wsb[:, p * C : (p + 1) * C],
                            rhs=gts[:],
                            start=(p == PAIR_LO[b]),
                            stop=(p == PAIR_HI[b] - 1),
                        )
                    xb = xbp.tile([64, 128], mybir.dt.float32, tag="xb")
                    nc.vector.tensor_copy(out=xb[:], in_=acc[:])
                    nc.sync.dma_start(out=xs[b][:, ds(i, 128)], in_=xb[:])

    nc.compile()
    return nc


_CACHE = {}


def _get_runner():
    if "run" in _CACHE:
        return _CACHE["run"]

    install_neuronx_cc_hook()
    nc = _build_bass()
    partition_name = nc.partition_id_tensor.name if nc.partition_id_tensor else None

    in_names, out_names, out_avals = [], [], []
    for alloc in nc.m.functions[0].allocations:
        if not isinstance(alloc, mybir.MemoryLocationSet):
            continue
        name = alloc.memorylocations[0].name
        if alloc.kind == "ExternalInput":
            if name != partition_name:
                in_names.append(name)
        elif alloc.kind == "ExternalOutput":
            out_names.append(name)
            out_avals.append(
                jax.core.ShapedArray(tuple(alloc.tensor_shape), mybir.dt.np(alloc.dtype))
            )
    all_in = list(in_names) + list(out_names)
    if partition_name is not None:
        all_in.append(partition_name)
    # bass inputs are fed by name; build kwargs order map
    assert set(in_names) == {"table", "wp", "idx"}, in_names
    assert out_names == ["x1", "x2", "x3"], out_names

    # --- jit 1: the bass custom call only (hook requires a pure module) ---
    def conv_body(*args):
        # args order must match in_names then the 3 output-zero buffers
        operands = list(args)
        if partition_name is not None:
            operands.append(partition_id_tensor())
        outs = _bass_exec_p.bind(
            *operands,
            out_avals=tuple(out_avals),
            in_names=tuple(all_in),
            out_names=tuple(out_names),
            lowering_input_output_aliases=(),
            sim_require_finite=True,
            sim_require_nnan=True,
            nc=nc,
        )
        return tuple(outs)

    devices = jax.devices()[:NCORES]
    mesh = Mesh(np.asarray(devices), ("core",))
    spec_by_name = {"table": P(), "wp": P(), "idx": P("core")}
    conv_in_specs = tuple(spec_by_name[n] for n in in_names) + (P("core"),) * 3
    conv_fn = jax.jit(
        shard_map(
            conv_body,
            mesh=mesh,
            in_specs=conv_in_specs,
            out_specs=(P("core"),) * 3,
            check_rep=False,
        ),
        donate_argnums=tuple(range(len(in_names), len(in_names) + 3)),
        keep_unused=True,
    )

    # --- jit 2: BN stats (psum over cores) + ReLU + branch sum + transpose ---
    def bn_body(x1, x2, x3, gs, bs):
        y = None
        inv_n = np.float32(1.0 / N)
        for b, xb in enumerate((x1, x2, x3)):
            s = jax.lax.psum(jnp.sum(xb, axis=1), "core") * inv_n        # [C]
            sq = jax.lax.psum(jnp.sum(xb * xb, axis=1), "core") * inv_n  # [C]
            var = sq - s * s
            scale = gs[b] * jax.lax.rsqrt(var + EPS)                     # [C]
            shift = bs[b] - s * scale
            yb = jax.nn.relu(xb * scale[:, None] + shift[:, None])
            y = yb if y is None else y + yb
        return jnp.transpose(y)  # [NC_SHARD, C]

    bn_fn = jax.jit(
        shard_map(
            bn_body,
            mesh=mesh,
            in_specs=(P("core"), P("core"), P("core"), P(), P()),
            out_specs=P("core"),
            check_rep=False,
        )
    )

    repl = NamedSharding(mesh, P())
    shard = NamedSharding(mesh, P("core"))

    def fn(table_d, wp_d, idx_d, gs_d, bs_d):
        by_name = {"table": table_d, "wp": wp_d, "idx": idx_d}
        zeros = [
            jax.device_put(np.zeros((NCORES * C, NC_SHARD), np.float32), shard)
            for _ in range(3)
        ]
        x1, x2, x3 = conv_fn(*[by_name[n] for n in in_names], *zeros)
        return bn_fn(x1, x2, x3, gs_d, bs_d)

    _CACHE["run"] = (fn, repl, shard)
    return _CACHE["run"]


def _prep_inputs(feats, W1, W2, W3, nbr1, nbr2, nbr3):
    table = np.concatenate([np.asarray(feats, np.float32), np.zeros((1, C), np.float32)], axis=0)

    # W pairs: pad each branch to even k count with zero weights, stack pairs
    w_all = np.concatenate(
        [
            np.concatenate([np.asarray(W, np.float32), np.zeros((kp - kr, C, C), np.float32)], axis=0)
            for W, kr, kp in ((W1, 27, 28), (W2, 125, 126), (W3, 343, 344))
        ],
        axis=0,
    )  # [KTOT, C, C]
    # wp[:, p*64:(p+1)*64] = vstack(w_all[2p], w_all[2p+1])  -> [128, NPAIR*C]
    wp = (
        w_all.reshape(NPAIR, 2, C, C)        # [p, j, cin, cout]
        .transpose(0, 1, 2, 3)
        .reshape(NPAIR, 2 * C, C)            # [p, 128, 64]
        .transpose(1, 0, 2)                  # [128, p, 64]
        .reshape(2 * C, NPAIR * C)
    ).copy()

    # index tensor: [KTOT, N] padded along k (value N) then padded along voxels
    nbrs = []
    for nbr, kr, kp in ((nbr1, 27, 28), (nbr2, 125, 126), (nbr3, 343, 344)):
        nbr = np.asarray(nbr, np.int32)
        if kp > kr:
            nbr = np.concatenate([nbr, np.full((kp - kr, N), N, np.int32)], axis=0)
        nbrs.append(nbr)
    idx_all = np.concatenate(nbrs, axis=0)  # [KTOT, N]
    pad_vox = NCORES * NC_SHARD - N
    idx_all = np.concatenate([idx_all, np.full((KTOT, pad_vox), N, np.int32)], axis=1)
    idx_t = np.ascontiguousarray(idx_all.T)  # [NCORES*NC_SHARD, KTOT]
    return table, wp, idx_t


def kernel(feats, W1, W2, W3, g1, b1, g2, b2, g3, b3, nbr1, nbr2, nbr3):
    fn, repl, shard = _get_runner()
    table, wp, idx_t = _prep_inputs(feats, W1, W2, W3, nbr1, nbr2, nbr3)
    gs = np.stack([np.asarray(g, np.float32) for g in (g1, g2, g3)])
    bs = np.stack([np.asarray(b, np.float32) for b in (b1, b2, b3)])

    table_d = jax.device_put(table, repl)
    wp_d = jax.device_put(wp, repl)
    idx_d = jax.device_put(idx_t, shard)
    gs_d = jax.device_put(gs, repl)
    bs_d = jax.device_put(bs, repl)

    out = fn(table_d, wp_d, idx_d, gs_d, bs_d)
    out = np.asarray(out)[:N]
    return out


# expose device-resident runner for timing in test.py
def _timed_call(args):
    fn, repl, shard = _get_runner()
    out = fn(*args)
    jax.block_until_ready(out)
    return out


def _place(feats, W1, W2, W3, g1, b1, g2, b2, g3, b3, nbr1, nbr2, nbr3):
    fn, repl, shard = _get_runner()
    table, wp, idx_t = _prep_inputs(feats, W1, W2, W3, nbr1, nbr2, nbr3)
    gs = np.stack([np.asarray(g, np.float32) for g in (g1, g2, g3)])
    bs = np.stack([np.asarray(b, np.float32) for b in (b1, b2, b3)])
    return (
        jax.device_put(table, repl),
        jax.device_put(wp, repl),
        jax.device_put(idx_t, shard),
        jax.device_put(gs, repl),
        jax.device_put(bs, repl),
    )


# revision 10
# speedup vs baseline: 1.6097x; 1.6097x over previous
"""Trainium2 Bass kernel for nn_ASPP (gather-GEMM sparse conv x3 + BN + ReLU + sum).

Architecture:
  - Shard voxels (rows of feats / columns of nbr maps) across 8 NeuronCores.
  - Per core, a Bass/Tile kernel computes the three sparse-conv outputs
    x_b = sum_k feats_pad[nbr_b[k, v]] @ W_b[k]  for its voxel shard,
    channel-major [64, Nc], using:
      * indirect DMA row-gathers from an HBM-resident feats_pad table
        (128 rows per call, int32 offsets),
      * PE transpose of gathered [128 vox, 2k x 64ch] pair-blocks,
      * paired matmuls accumulating in PSUM per branch.
  - BatchNorm statistics (mean/var over all N voxels) + ReLU + branch sum are
    done by XLA around the Bass custom call, with jax.lax.psum across the 8
    cores inside one shard_map jit (single device round trip).

The kernel is built and jitted once per process and cached.
"""

import numpy as np

import jax
import jax.numpy as jnp
from jax.sharding import Mesh, PartitionSpec as P, NamedSharding
from jax.experimental.shard_map import shard_map

import concourse.bass as bass
import concourse.mybir as mybir
import concourse.tile as tile
from concourse import bacc
from concourse.bass import ds
from concourse.bass2jax import _bass_exec_p, install_neuronx_cc_hook, partition_id_tensor
from concourse.masks import make_identity

NCORES = 8
N = 100000
C = 64
EPS = 1e-5

# branch k-counts padded to even so 2-k pairs never straddle a branch
K_RAW = (27, 125, 343)
K_PAD = (28, 126, 344)
KTOT = sum(K_PAD)            # 498
NPAIR = KTOT // 2            # 249
# pair index ranges per branch: [0,14), [14,77), [77,249)
PAIR_LO = (0, K_PAD[0] // 2, (K_PAD[0] + K_PAD[1]) // 2)
PAIR_HI = (K_PAD[0] // 2, (K_PAD[0] + K_PAD[1]) // 2, NPAIR)

NC_SHARD = 12544             # per-core voxels (98 * 128); 8*12544 = 100352 >= N
NTILE = NC_SHARD // 128      # 98

import os
GATHER_BF16 = os.environ.get("ASPP_BF16", "") != ""  # f32 default (bf16 PSUM transpose crashes)
GDT = None  # set in _build_bass
GNP = None


def _indirect_gather_q(eng, out, in_, in_offset, queue_name):
    """nc.gpsimd.indirect_dma_start (gather form), with a selectable SWDGE
    queue so calls can spread across the 4 Q7 descriptor-gen contexts."""
    offset_ap = in_offset.ap
    offset_axis = in_offset.axis
    assert in_.space == bass.MemorySpace.DRAM
    assert out.space == bass.MemorySpace.SBUF
    src_ap = in_
    assert isinstance(src_ap.offset, int) and src_ap.offset == 0
    out_ap = eng.lower_ap_dma(out, for_indirect_dma=True)
    in_ap = eng.lower_ap_dma(in_, for_indirect_dma=True)
    assert len(in_ap) == 1 and len(out_ap) == 1
    offset_ap_l = eng.lower_ap_dma(offset_ap)
    assert len(offset_ap_l) == 1
    in_ap.append(offset_ap_l[0])

    ap_shape = src_ap.shape
    coef = 1
    for i in range(offset_axis + 1, len(ap_shape)):
        coef *= ap_shape[i]
    dynamic_ap_info = mybir.DynamicAccessPatternInfo(
        c=0,
        actual_ap=out.ap,
        indirect_dim_max_index=ap_shape[offset_axis],
        offset_expr=[
            mybir.DynamicAccessPatternOffsetExpr(
                coef=coef,
                aff_expr=mybir.DynamicAccessPatternOffsetExprAffExpr(
                    kind="IndirectArgId", arg_id=1
                ),
            )
        ],
    )
    in_ap[0].dynamic_ap_info = dynamic_ap_info
    return eng.add_instruction(
        mybir.InstDMACopy(
            name=eng.bass.get_next_instruction_name(),
            queue=queue_name,
            mode="Copy",
            ins=in_ap,
            outs=out_ap,
            oob_is_err=True,
            cce_op=mybir.AluOpType.bypass,
        )
    )


NQUEUES = int(os.environ.get("ASPP_NQ", "4"))


def _build_bass():
    """Per-core conv kernel: inputs table [N+1, C] f32, wp [128, NPAIR*64] f32,
    idx [NC_SHARD, KTOT] int32; outputs x1, x2, x3 [64, NC_SHARD] f32."""
    nc = bacc.Bacc(
        "TRN2", target_bir_lowering=False, debug=False, num_devices=NCORES,
        num_swdge_queues=NQUEUES, dynamic_dma_scratch_size=65536,
    )

    gdt = mybir.dt.bfloat16 if GATHER_BF16 else mybir.dt.float32
    table = nc.dram_tensor("table", [N + 1, C], gdt, kind="ExternalInput")
    wp = nc.dram_tensor("wp", [128, NPAIR * C], gdt, kind="ExternalInput")
    idx = nc.dram_tensor("idx", [NC_SHARD, KTOT], mybir.dt.int32, kind="ExternalInput")
    xs = [
        nc.dram_tensor(f"x{b+1}", [C, NC_SHARD], mybir.dt.float32, kind="ExternalOutput")
        for b in range(3)
    ]

    with tile.TileContext(nc) as tc:
        with (
            tc.tile_pool(name="const", bufs=1) as constp,
            tc.tile_pool(name="idxp", bufs=2) as idxp,
            tc.tile_pool(name="gp", bufs=40) as gpool,
            tc.tile_pool(name="gtp", bufs=8) as gtp,
            tc.tile_pool(name="xbp", bufs=4) as xbp,
            tc.tile_pool(name="ps", bufs=3, space="PSUM") as psp,
            tc.tile_pool(name="accp", bufs=3, space="PSUM") as accp,
        ):
            wsb = constp.tile([128, NPAIR * C], gdt)
            nc.sync.dma_start(out=wsb[:], in_=wp[:])
            ident = constp.tile([128, 128], gdt)
            make_identity(nc, ident[:])

            with tc.For_i(
                0, NC_SHARD, 128,
                hint_engines=(mybir.EngineType.Pool, mybir.EngineType.PE),
            ) as i:
                idx_t = idxp.tile([128, KTOT], mybir.dt.int32, tag="idx")
                nc.sync.dma_start(out=idx_t[:], in_=idx[ds(i, 128)])

                for b in range(3):
                    acc = accp.tile([64, 128], mybir.dt.float32, tag="acc")
                    for p in range(PAIR_LO[b], PAIR_HI[b]):
                        gt = gpool.tile([128, 128], gdt, tag="g")
                        qn0 = (2 * p) % NQUEUES
                        qn1 = (2 * p + 1) % NQUEUES
                        _indirect_gather_q(
                            nc.gpsimd, gt[:, 0:C], table[:],
                            bass.IndirectOffsetOnAxis(ap=idx_t[:, 2 * p : 2 * p + 1], axis=0),
                            f"qPoolDynamic{qn0 or ''}",
                        )
                        _indirect_gather_q(
                            nc.gpsimd, gt[:, C : 2 * C], table[:],
                            bass.IndirectOffsetOnAxis(ap=idx_t[:, 2 * p + 1 : 2 * p + 2], axis=0),
                            f"qPoolDynamic{qn1 or ''}",
                        )
                        tp = psp.tile([128, 128], gdt, tag="tp")
                        nc.tensor.transpose(out=tp[:], in_=gt[:], identity=ident[:])
                        gts = gtp.tile([128, 128], gdt, tag="gts")
                        if p % 2 == 0:
                            nc.vector.tensor_copy(out=gts[:], in_=tp[:])
                        else:
                            nc.scalar.copy(out=gts[:], in_=tp[:])
                        nc.tensor.matmul(
                            out=acc[:],
                            lhsT=wsb[:, p * C : (p + 1) * C],
                            rhs=gts[:],
                            start=(p == PAIR_LO[b]),
                            stop=(p == PAIR_HI[b] - 1),
                        )
                    xb = xbp.tile([64, 128], mybir.dt.float32, tag="xb")
                    nc.vector.tensor_copy(out=xb[:], in_=acc[:])
                    nc.sync.dma_start(out=xs[b][:, ds(i, 128)], in_=xb[:])

    nc.compile()
    return nc


_CACHE = {}


def _get_runner():
    if "run" in _CACHE:
        return _CACHE["run"]

    install_neuronx_cc_hook()
    nc = _build_bass()
    partition_name = nc.partition_id_tensor.name if nc.partition_id_tensor else None

    in_names, out_names, out_avals = [], [], []
    for alloc in nc.m.functions[0].allocations:
        if not isinstance(alloc, mybir.MemoryLocationSet):
            continue
        name = alloc.memorylocations[0].name
        if alloc.kind == "ExternalInput":
            if name != partition_name:
                in_names.append(name)
        elif alloc.kind == "ExternalOutput":
            out_names.append(name)
            out_avals.append(
                jax.core.ShapedArray(tuple(alloc.tensor_shape), mybir.dt.np(alloc.dtype))
            )
    all_in = list(in_names) + list(out_names)
    if partition_name is not None:
        all_in.append(partition_name)
    # bass inputs are fed by name; build kwargs order map
    assert set(in_names) == {"table", "wp", "idx"}, in_names
    assert out_names == ["x1", "x2", "x3"], out_names

    # --- jit 1: the bass custom call only (hook requires a pure module) ---
    def conv_body(*args):
        # args order must match in_names then the 3 output-zero buffers
        operands = list(args)
        if partition_name is not None:
            operands.append(partition_id_tensor())
        outs = _bass_exec_p.bind(
            *operands,
            out_avals=tuple(out_avals),
            in_names=tuple(all_in),
            out_names=tuple(out_names),
            lowering_input_output_aliases=(),
            sim_require_finite=True,
            sim_require_nnan=True,
            nc=nc,
        )
        return tuple(outs)

    devices = jax.devices()[:NCORES]
    mesh = Mesh(np.asarray(devices), ("core",))
    spec_by_name = {"table": P(), "wp": P(), "idx": P("core")}
    conv_in_specs = tuple(spec_by_name[n] for n in in_names) + (P("core"),) * 3
    conv_fn = jax.jit(
        shard_map(
            conv_body,
            mesh=mesh,
            in_specs=conv_in_specs,
            out_specs=(P("core"),) * 3,
            check_rep=False,
        ),
        donate_argnums=tuple(range(len(in_names), len(in_names) + 3)),
        keep_unused=True,
    )

    # --- jit 2: BN stats (psum over cores) + ReLU + branch sum + transpose ---
    def bn_body(x1, x2, x3, gs, bs):
        y = None
        inv_n = np.float32(1.0 / N)
        for b, xb in enumerate((x1, x2, x3)):
            s = jax.lax.psum(jnp.sum(xb, axis=1), "core") * inv_n        # [C]
            sq = jax.lax.psum(jnp.sum(xb * xb, axis=1), "core") * inv_n  # [C]
            var = sq - s * s
            scale = gs[b] * jax.lax.rsqrt(var + EPS)                     # [C]
            shift = bs[b] - s * scale
            yb = jax.nn.relu(xb * scale[:, None] + shift[:, None])
            y = yb if y is None else y + yb
        return jnp.transpose(y)  # [NC_SHARD, C]

    bn_fn = jax.jit(
        shard_map(
            bn_body,
            mesh=mesh,
            in_specs=(P("core"), P("core"), P("core"), P(), P()),
            out_specs=P("core"),
            check_rep=False,
        )
    )

    repl = NamedSharding(mesh, P())
    shard = NamedSharding(mesh, P("core"))

    def fn(table_d, wp_d, idx_d, gs_d, bs_d):
        by_name = {"table": table_d, "wp": wp_d, "idx": idx_d}
        zeros = [
            jax.device_put(np.zeros((NCORES * C, NC_SHARD), np.float32), shard)
            for _ in range(3)
        ]
        x1, x2, x3 = conv_fn(*[by_name[n] for n in in_names], *zeros)
        return bn_fn(x1, x2, x3, gs_d, bs_d)

    _CACHE["run"] = (fn, repl, shard)
    return _CACHE["run"]


def _prep_inputs(feats, W1, W2, W3, nbr1, nbr2, nbr3):
    import ml_dtypes
    gnp = ml_dtypes.bfloat16 if GATHER_BF16 else np.float32
    table = np.concatenate([np.asarray(feats, np.float32), np.zeros((1, C), np.float32)], axis=0).astype(gnp)

    # W pairs: pad each branch to even k count with zero weights, stack pairs
    w_all = np.concatenate(
        [
            np.concatenate([np.asarray(W, np.float32), np.zeros((kp - kr, C, C), np.float32)], axis=0)
            for W, kr, kp in ((W1, 27, 28), (W2, 125, 126), (W3, 343, 344))
        ],
        axis=0,
    )  # [KTOT, C, C]
    # wp[:, p*64:(p+1)*64] = vstack(w_all[2p], w_all[2p+1])  -> [128, NPAIR*C]
    wp = (
        w_all.reshape(NPAIR, 2, C, C)        # [p, j, cin, cout]
        .transpose(0, 1, 2, 3)
        .reshape(NPAIR, 2 * C, C)            # [p, 128, 64]
        .transpose(1, 0, 2)                  # [128, p, 64]
        .reshape(2 * C, NPAIR * C)
    ).astype(gnp).copy()

    # index tensor: [KTOT, N] padded along k (value N) then padded along voxels
    nbrs = []
    for nbr, kr, kp in ((nbr1, 27, 28), (nbr2, 125, 126), (nbr3, 343, 344)):
        nbr = np.asarray(nbr, np.int32)
        if kp > kr:
            nbr = np.concatenate([nbr, np.full((kp - kr, N), N, np.int32)], axis=0)
        nbrs.append(nbr)
    idx_all = np.concatenate(nbrs, axis=0)  # [KTOT, N]
    pad_vox = NCORES * NC_SHARD - N
    idx_all = np.concatenate([idx_all, np.full((KTOT, pad_vox), N, np.int32)], axis=1)
    idx_t = np.ascontiguousarray(idx_all.T)  # [NCORES*NC_SHARD, KTOT]
    return table, wp, idx_t


def kernel(feats, W1, W2, W3, g1, b1, g2, b2, g3, b3, nbr1, nbr2, nbr3):
    fn, repl, shard = _get_runner()
    table, wp, idx_t = _prep_inputs(feats, W1, W2, W3, nbr1, nbr2, nbr3)
    gs = np.stack([np.asarray(g, np.float32) for g in (g1, g2, g3)])
    bs = np.stack([np.asarray(b, np.float32) for b in (b1, b2, b3)])

    table_d = jax.device_put(table, repl)
    wp_d = jax.device_put(wp, repl)
    idx_d = jax.device_put(idx_t, shard)
    gs_d = jax.device_put(gs, repl)
    bs_d = jax.device_put(bs, repl)

    out = fn(table_d, wp_d, idx_d, gs_d, bs_d)
    out = np.asarray(out)[:N]
    return out


# expose device-resident runner for timing in test.py
def _timed_call(args):
    fn, repl, shard = _get_runner()
    out = fn(*args)
    jax.block_until_ready(out)
    return out


def _place(feats, W1, W2, W3, g1, b1, g2, b2, g3, b3, nbr1, nbr2, nbr3):
    fn, repl, shard = _get_runner()
    table, wp, idx_t = _prep_inputs(feats, W1, W2, W3, nbr1, nbr2, nbr3)
    gs = np.stack([np.asarray(g, np.float32) for g in (g1, g2, g3)])
    bs = np.stack([np.asarray(b, np.float32) for b in (b1, b2, b3)])
    return (
        jax.device_put(table, repl),
        jax.device_put(wp, repl),
        jax.device_put(idx_t, shard),
        jax.device_put(gs, repl),
        jax.device_put(bs, repl),
    )
